# revision 5
# baseline (speedup 1.0000x reference)
"""Grouped MoE MLP (SwiGLU) kernel for Trainium2, 8 NeuronCores.

Strategy (load-balanced expert-parallel):
  The per-expert token counts are ragged (max 3072 vs mean 2048), so the
  baseline one-expert-per-core split leaves the hot core with 1.5x the
  average work -- and the trace shows TensorE is 96% busy, i.e. the
  kernel is at the matmul roofline for whatever token count the hot
  core carries.  The only lever is balance.

  Tokens are split into NT=256-token chunks (every chunk belongs to one
  expert; tokens arrive pre-sorted by expert).  The 64 chunks are packed
  into 8 cores x fixed per-core "slots" (e.g. sizes (3,3,2) chunks): one
  slot processes chunks of a single expert, so each core runs exactly
  sum(S) chunks = 2048 tokens.  A small backtracking packer finds a
  zero-waste structure for the given counts (for the reference counts
  the (3,3,2) packing is exact).

  Per-slot weights are streamed from HBM: GEMM1 weights at (gate,up)
  column-group granularity through a small SBUF ring (a group is dead
  once its GEMM1 finishes, so the full 11.5 MiB never sits in SBUF),
  GEMM2 weights one 5.8 MiB block per slot, double-use-free.  DMA per
  core totals ~69 MiB against ~190 us of bandwidth -- well hidden under
  ~460 us of matmul work.

  Device program per slot (dense SwiGLU over the slot's chunks):
    GEMM1 group-major: for mp in 0..10: stream w1[mp] (gate+up), then
      for each chunk: accumulate 16 k-tiles into PSUM for gate and up,
      SiLU (ACT) * up (DVE) -> h[:, mp, :] in SBUF (bf16)
    GEMM2 per chunk: tokens on the PSUM partition dim, out lands in
      natural [T, HIDDEN] layout.

  All device I/O is bf16 (cast on host) to halve staged bytes.
"""

import math
from contextlib import ExitStack

import ml_dtypes
import numpy as np

P = 128
HIDDEN = 2048
INTER = 1408
GU = 2 * INTER            # 2816 gate+up columns
KH = HIDDEN // P          # 16 k-tiles for GEMM1
KI = INTER // P           # 11 feature groups / GEMM2 k-tiles
NO = HIDDEN // 512        # 4 output column blocks of 512
N_CORES = 8
NT = 256                  # tokens per chunk
TB = NT // P              # 128-token blocks per chunk (2)

BF16 = ml_dtypes.bfloat16

_PROGRAM_CACHE: dict = {}


# --------------------------------------------------------------------------
# chunk -> slot packing
# --------------------------------------------------------------------------

def _structures(c):
    """All descending partitions of c into 1..4 parts of size <= 6."""
    out = []

    def rec(rem, maxp, cur):
        if rem == 0:
            out.append(tuple(cur))
            return
        if len(cur) == 4:
            return
        for p in range(min(maxp, rem), 0, -1):
            rec(rem - p, p, cur + [p])

    rec(c, min(c, 6), [])
    out.sort(key=lambda s: (len(s), -min(s)))
    return out


def _pack(m, S):
    """Pack expert chunk-counts m into 8 copies of slot structure S.

    Returns {(core, slot_idx): (expert, n_chunks)} or None.  A slot holds
    chunks of a single expert and may be partially filled (padding)."""
    slots = []
    for si, sz in enumerate(S):
        for core in range(N_CORES):
            slots.append((sz, core, si))
    slots.sort(key=lambda t: -t[0])
    rem = list(m)
    assign = {}
    nodes = [0]

    def feasible(i):
        caps = [s[0] for s in slots[i:]]
        need = [r for r in rem if r > 0]
        if not need:
            return True
        if not caps or sum(caps) < sum(need):
            return False
        mx = max(caps)
        return sum((r + mx - 1) // mx for r in need) <= len(caps)

    def rec(i):
        nodes[0] += 1
        if nodes[0] > 300000:
            return False
        if all(r == 0 for r in rem):
            return True
        if i == len(slots) or not feasible(i):
            return False
        sz, core, si = slots[i]
        cands = sorted(
            (e for e in range(len(rem)) if rem[e] > 0),
            key=lambda e: (rem[e] != sz, -rem[e]),
        )
        tried = set()
        for e in cands:
            amt = min(rem[e], sz)
            if amt in tried:
                continue
            tried.add(amt)
            rem[e] -= amt
            assign[(core, si)] = (e, amt)
            if rec(i + 1):
                return True
            del assign[(core, si)]
            rem[e] += amt
        return rec(i + 1)  # leave this slot empty

    return assign if rec(0) else None


def _plan(counts):
    """-> (S, cores) where cores[r] = [(expert|None, echunk0, n_real), ...]
    one entry per slot of S."""
    m = [(c + NT - 1) // NT for c in counts]
    total = sum(m)
    base = (total + N_CORES - 1) // N_CORES
    for c in range(base, base + 9):
        for S in _structures(c):
            asg = _pack(list(m), S)
            if asg is None:
                continue
            # hand out chunk ranges per expert in deterministic slot order
            slots = []
            for si, sz in enumerate(S):
                for core in range(N_CORES):
                    slots.append((sz, core, si))
            slots.sort(key=lambda t: -t[0])
            nxt = [0] * len(m)
            cores = [[None] * len(S) for _ in range(N_CORES)]
            for sz, core, si in slots:
                ent = asg.get((core, si))
                if ent is None:
                    continue
                e, amt = ent
                cores[core][si] = (e, nxt[e], amt)
                nxt[e] += amt
            return S, cores
    raise RuntimeError(f"no packing found for counts {counts}")


# --------------------------------------------------------------------------
# device program
# --------------------------------------------------------------------------

def _build_program(S):
    import concourse.mybir as mybir
    import concourse.tile as tile
    from concourse import bacc

    bf16 = mybir.dt.bfloat16
    f32 = mybir.dt.float32

    n_slots = len(S)
    n_chunks = sum(S)

    nc = bacc.Bacc(None, target_bir_lowering=False, debug=False)
    # x: chunk-major, hidden on partitions; each chunk one contiguous 1 MiB DMA
    xT = nc.dram_tensor("xT", [n_chunks, P, KH, NT], bf16, kind="ExternalInput")
    # w1: per (slot, group): [P, 2(gate/up), KH, P] contiguous 1 MiB blocks
    w1 = nc.dram_tensor(
        "w1", [n_slots, KI, P, 2, KH, P], bf16, kind="ExternalInput"
    )
    # w2: per slot: [P, KI, HIDDEN] contiguous 5.5 MiB block
    w2 = nc.dram_tensor("w2", [n_slots, P, KI, HIDDEN], bf16, kind="ExternalInput")
    out = nc.dram_tensor(
        "out", [n_chunks, TB, NO, P, 512], bf16, kind="ExternalOutput"
    )

    with tile.TileContext(nc) as tc, ExitStack() as ctx:
        w1_pool = ctx.enter_context(tc.tile_pool(name="w1p", bufs=6))
        w2_pool = ctx.enter_context(tc.tile_pool(name="w2p", bufs=1))
        x_pool = ctx.enter_context(tc.tile_pool(name="xp", bufs=6))
        h_pool = ctx.enter_context(tc.tile_pool(name="hp", bufs=6))
        g_pool = ctx.enter_context(tc.tile_pool(name="gp", bufs=3))
        o_pool = ctx.enter_context(tc.tile_pool(name="op", bufs=4))
        ps1 = ctx.enter_context(tc.tile_pool(name="ps1", bufs=2, space="PSUM"))
        ps2 = ctx.enter_context(tc.tile_pool(name="ps2", bufs=3, space="PSUM"))

        g0 = 0
        for si, sz in enumerate(S):
            # ---- DMA emission for this slot ----
            # order on the sync ring: x chunk0, w1 group0, rest of x, then
            # w1 groups 1..10 (first matmul gates on ~2 MiB only).
            # w2 + out stores ride the scalar ring so they never head-of-
            # line block the sync ring.
            xts = []
            w1ts = []
            for j in range(sz):
                t = x_pool.tile([P, KH, NT], bf16, tag="xt")
                if si == 0 and j == 0:
                    # split the very first loads so the first matmul chain
                    # gates on ~0.8 MiB instead of 2.1 MiB
                    nc.sync.dma_start(t[:, 0:4], xT[0, :, 0:4])
                    t0 = w1_pool.tile([P, 2, KH, P], bf16, tag="w1g")
                    nc.sync.dma_start(t0[:, 0], w1[0, 0, :, 0])
                    w1ts.append(t0)
                    nc.sync.dma_start(t[:, 4:8], xT[0, :, 4:8])
                    nc.sync.dma_start(t[:, 8:12], xT[0, :, 8:12])
                    nc.sync.dma_start(t[:, 12:16], xT[0, :, 12:16])
                    nc.sync.dma_start(t0[:, 1], w1[0, 0, :, 1])
                else:
                    nc.sync.dma_start(t[:], xT[g0 + j])
                    if j == 0:
                        t0 = w1_pool.tile([P, 2, KH, P], bf16, tag="w1g")
                        nc.sync.dma_start(t0[:], w1[si, 0])
                        w1ts.append(t0)
                xts.append(t)
            for mp in range(1, KI):
                t = w1_pool.tile([P, 2, KH, P], bf16, tag="w1g")
                nc.sync.dma_start(t[:], w1[si, mp])
                w1ts.append(t)
            w2t = w2_pool.tile([P, KI, HIDDEN], bf16, tag="w2t")

            # ---- GEMM1 (group-major over the slot's chunks) ----
            hts = []
            for _ in range(sz):
                ht = h_pool.tile([P, KI, NT], bf16, tag="ht")
                hts.append(ht)
            for mp in range(KI):
                w1t = w1ts[mp]
                for j in range(sz):
                    pg = ps1.tile([P, NT], f32, tag="pg")
                    pu = ps1.tile([P, NT], f32, tag="pu")
                    for k in range(KH):
                        nc.tensor.matmul(
                            pg[:],
                            w1t[:, 0, k],
                            xts[j][:, k],
                            start=(k == 0),
                            stop=(k == KH - 1),
                        )
                    for k in range(KH):
                        nc.tensor.matmul(
                            pu[:],
                            w1t[:, 1, k],
                            xts[j][:, k],
                            start=(k == 0),
                            stop=(k == KH - 1),
                        )
                    gt = g_pool.tile([P, NT], bf16, tag="gt")
                    nc.scalar.activation(
                        gt[:], pg[:], mybir.ActivationFunctionType.Silu
                    )
                    nc.vector.tensor_mul(hts[j][:, mp], gt[:], pu[:])
                # one w2 column-group per GEMM1 group, emitted on the scalar
                # queue AFTER this group's silu: ACT's FIFO ordering paces the
                # 5.5 MiB w2 load across GEMM1 instead of letting it hog HBM
                # bandwidth up front (it front-ran the critical x/w1 loads
                # and cost a 21 us startup stall when issued in one piece).
                nc.scalar.dma_start(w2t[:, mp], w2[si, :, mp])

            # ---- GEMM2 (tokens on PSUM partitions) ----
            for j in range(sz):
                for tb in range(TB):
                    for m in range(NO):
                        po = ps2.tile([P, 512], f32, tag="po")
                        for k in range(KI):
                            nc.tensor.matmul(
                                po[:],
                                hts[j][:, k, tb * P : (tb + 1) * P],
                                w2t[:, k, m * 512 : (m + 1) * 512],
                                start=(k == 0),
                                stop=(k == KI - 1),
                            )
                        om = o_pool.tile([P, 512], bf16, tag="om")
                        nc.vector.tensor_copy(om[:], po[:])
                        nc.scalar.dma_start(out[g0 + j, tb, m], om[:])
            g0 += sz
    nc.compile()
    return nc


def _get_program(S):
    if S not in _PROGRAM_CACHE:
        _PROGRAM_CACHE[S] = _build_program(S)
    return _PROGRAM_CACHE[S]


# --------------------------------------------------------------------------
# host-side pack / unpack
# --------------------------------------------------------------------------

def _pack_w1(w: np.ndarray) -> np.ndarray:
    # [HIDDEN, GU] f32 -> [KI, P, 2, KH, P] bf16  (h = 128k + p, c = g*INTER
    # + 128*mp + j)
    return np.ascontiguousarray(
        w.reshape(KH, P, 2, KI, P).transpose(3, 1, 2, 0, 4)
    ).astype(BF16)


def _pack_w2(w: np.ndarray) -> np.ndarray:
    # [INTER, HIDDEN] f32 -> [P, KI, HIDDEN] bf16
    return np.ascontiguousarray(
        w.reshape(KI, P, HIDDEN).transpose(1, 0, 2)
    ).astype(BF16)


def _run(
    hidden_states: np.ndarray,
    merged_gate_up_proj: np.ndarray,
    merged_down_proj: np.ndarray,
    num_tokens_per_expert: np.ndarray,
    trace: bool = False,
):
    counts = [int(c) for c in np.asarray(num_tokens_per_expert)]
    n_exp = len(counts)
    offs = np.concatenate([[0], np.cumsum(counts)]).astype(int)
    total = int(offs[-1])

    S, cores = _plan(counts)
    n_slots = len(S)
    n_chunks = sum(S)
    slot_base = np.concatenate([[0], np.cumsum(S)]).astype(int)

    nc = _get_program(S)

    from concurrent.futures import ThreadPoolExecutor

    pool = ThreadPoolExecutor(8)

    # [TOTAL, HIDDEN] f32 -> bf16 -> [P, KH, total] transposed view source
    x_bf16 = hidden_states[:total].astype(BF16)
    xT_full = np.empty((HIDDEN, total), dtype=BF16)

    def _tr(k):
        xT_full[k * P : (k + 1) * P] = x_bf16[:, k * P : (k + 1) * P].T

    list(pool.map(_tr, range(KH)))
    xT_pkt = xT_full.reshape(KH, P, total).transpose(1, 0, 2)  # [P, KH, total]

    w1_packed = list(pool.map(
        lambda e: _pack_w1(merged_gate_up_proj[e]), range(n_exp)
    ))
    w2_packed = list(pool.map(
        lambda e: _pack_w2(merged_down_proj[e]), range(n_exp)
    ))

    def _core_inputs(r):
        xc = np.zeros((n_chunks, P, KH, NT), dtype=BF16)
        w1c = np.empty((n_slots, KI, P, 2, KH, P), dtype=BF16)
        w2c = np.empty((n_slots, P, KI, HIDDEN), dtype=BF16)
        for si in range(n_slots):
            ent = cores[r][si]
            e = ent[0] if ent is not None else 0
            w1c[si] = w1_packed[e]
            w2c[si] = w2_packed[e]
            if ent is None:
                continue
            e, k0, amt = ent
            for j in range(amt):
                t0 = (k0 + j) * NT
                n = min(NT, counts[e] - t0)
                if n <= 0:
                    break
                xc[slot_base[si] + j, :, :, :n] = xT_pkt[
                    :, :, offs[e] + t0 : offs[e] + t0 + n
                ]
        return {"xT": xc, "w1": w1c, "w2": w2c}

    in_maps = list(pool.map(_core_inputs, range(N_CORES)))
    pool.shutdown(wait=True)

    res = _execute(nc, in_maps, trace)

    out = np.empty((total, HIDDEN), dtype=np.float32)

    def _unshard(r):
        o = res.results[r]["out"]  # [n_chunks, TB, NO, P, 512] bf16
        o = o.transpose(0, 1, 3, 2, 4).reshape(n_chunks, NT, HIDDEN)
        for si in range(n_slots):
            ent = cores[r][si]
            if ent is None:
                continue
            e, k0, amt = ent
            for j in range(amt):
                t0 = (k0 + j) * NT
                n = min(NT, counts[e] - t0)
                if n <= 0:
                    break
                out[offs[e] + t0 : offs[e] + t0 + n] = o[
                    slot_base[si] + j, :n
                ].astype(np.float32)

    upool = ThreadPoolExecutor(8)
    list(upool.map(_unshard, range(N_CORES)))
    upool.shutdown(wait=True)
    return out, res


# --------------------------------------------------------------------------
# execution (pjrt fast path with on-device zero outputs, axon fallback)
# --------------------------------------------------------------------------

def _execute(nc, in_maps, trace):
    from concourse.bass_utils import run_bass_kernel_spmd

    if not trace:
        try:
            return _execute_pjrt_dev_zeros(nc, in_maps)
        except Exception:
            pass
    return run_bass_kernel_spmd(
        nc, in_maps, list(range(N_CORES)), trace=trace
    )


_EXEC_CACHE: dict = {}


def _build_pjrt_executor(nc):
    from concourse.bass_utils import axon_active
    import concourse.mybir as mybir
    from concourse import bass2jax
    import jax
    import jax.numpy as jnp
    from jax.sharding import Mesh, PartitionSpec, NamedSharding
    from jax.experimental.shard_map import shard_map

    if not axon_active():
        raise RuntimeError("pjrt path requires axon")
    if nc.dbg_addr is not None:
        raise RuntimeError("debug program")

    bass2jax.install_neuronx_cc_hook()

    partition_name = nc.partition_id_tensor.name if nc.partition_id_tensor else None
    in_names, out_names, out_avals = [], [], []
    for alloc in nc.m.functions[0].allocations:
        if not isinstance(alloc, mybir.MemoryLocationSet):
            continue
        name = alloc.memorylocations[0].name
        if alloc.kind == "ExternalInput":
            if name != partition_name:
                in_names.append(name)
        elif alloc.kind == "ExternalOutput":
            out_names.append(name)
            out_avals.append(
                jax.core.ShapedArray(
                    tuple(alloc.tensor_shape), mybir.dt.np(alloc.dtype)
                )
            )
    n_params = len(in_names)
    n_outs = len(out_avals)
    all_names = in_names + out_names
    if partition_name is not None:
        all_names = all_names + [partition_name]
    donate = tuple(range(n_params, n_params + n_outs))

    def _body(*args):
        operands = list(args)
        if partition_name is not None:
            operands.append(bass2jax.partition_id_tensor())
        outs = bass2jax._bass_exec_p.bind(
            *operands,
            out_avals=tuple(out_avals),
            in_names=tuple(all_names),
            out_names=tuple(out_names),
            lowering_input_output_aliases=(),
            sim_require_finite=True,
            sim_require_nnan=True,
            nc=nc,
        )
        return tuple(outs)

    devices = jax.devices()[:N_CORES]
    assert len(devices) == N_CORES
    mesh = Mesh(np.asarray(devices), ("core",))
    in_specs = (PartitionSpec("core"),) * (n_params + n_outs)
    out_specs = (PartitionSpec("core"),) * n_outs
    sharded = jax.jit(
        shard_map(
            _body, mesh=mesh, in_specs=in_specs, out_specs=out_specs,
            check_rep=False,
        ),
        donate_argnums=donate,
        keep_unused=True,
    )
    zsharding = NamedSharding(mesh, PartitionSpec("core"))
    zero_fns = [
        jax.jit(
            lambda s=av.shape, d=av.dtype: jnp.zeros(
                (N_CORES * s[0], *s[1:]), d
            ),
            out_shardings=zsharding,
        )
        for av in out_avals
    ]
    return {
        "sharded": sharded,
        "zero_fns": zero_fns,
        "in_names": in_names,
        "out_names": out_names,
        "out_avals": out_avals,
    }


def _execute_pjrt_dev_zeros(nc, in_maps):
    """run_bass_via_pjrt equivalent with donated zero output buffers created
    on-device instead of staged from host numpy."""
    from concourse.bass_utils import BassKernelResults

    key = id(nc)
    if key not in _EXEC_CACHE:
        _EXEC_CACHE[key] = _build_pjrt_executor(nc)
    ex = _EXEC_CACHE[key]

    concat_in = [
        np.concatenate([np.asarray(m[name]) for m in in_maps], axis=0)
        for name in ex["in_names"]
    ]
    dev_zeros = [fn() for fn in ex["zero_fns"]]
    out_arrs = ex["sharded"](*concat_in, *dev_zeros)
    out_avals = ex["out_avals"]
    results = [
        {
            name: np.asarray(out_arrs[i]).reshape(
                N_CORES, *out_avals[i].shape
            )[c]
            for i, name in enumerate(ex["out_names"])
        }
        for c in range(N_CORES)
    ]
    return BassKernelResults(
        results=results,
        instructions_and_trace=None,
        profile_json=None,
        exec_time_ns=None,
    )


def kernel(**inputs) -> np.ndarray:
    return _run(**inputs, trace=False)[0]


def run_traced(**inputs):
    return _run(**inputs, trace=True)


# revision 7
# speedup vs baseline: 1.0035x; 1.0035x over previous
"""Grouped MoE MLP (SwiGLU) kernel for Trainium2, 8 NeuronCores.

Strategy (load-balanced expert-parallel):
  The per-expert token counts are ragged (max 3072 vs mean 2048), so the
  baseline one-expert-per-core split leaves the hot core with 1.5x the
  average work -- and the trace shows TensorE is 96% busy, i.e. the
  kernel is at the matmul roofline for whatever token count the hot
  core carries.  The only lever is balance.

  Tokens are split into NT=256-token chunks (every chunk belongs to one
  expert; tokens arrive pre-sorted by expert).  The 64 chunks are packed
  into 8 cores x fixed per-core "slots" (e.g. sizes (3,3,2) chunks): one
  slot processes chunks of a single expert, so each core runs exactly
  sum(S) chunks = 2048 tokens.  A small backtracking packer finds a
  zero-waste structure for the given counts (for the reference counts
  the (3,3,2) packing is exact).

  Per-slot weights are streamed from HBM: GEMM1 weights at (gate,up)
  column-group granularity through a small SBUF ring (a group is dead
  once its GEMM1 finishes, so the full 11.5 MiB never sits in SBUF),
  GEMM2 weights one 5.8 MiB block per slot, double-use-free.  DMA per
  core totals ~69 MiB against ~190 us of bandwidth -- well hidden under
  ~460 us of matmul work.

  Device program per slot (dense SwiGLU over the slot's chunks):
    GEMM1 group-major: for mp in 0..10: stream w1[mp] (gate+up), then
      for each chunk: accumulate 16 k-tiles into PSUM for gate and up,
      SiLU (ACT) * up (DVE) -> h[:, mp, :] in SBUF (bf16)
    GEMM2 per chunk: tokens on the PSUM partition dim, out lands in
      natural [T, HIDDEN] layout.

  All device I/O is bf16 (cast on host) to halve staged bytes.
"""

import math
from contextlib import ExitStack

import ml_dtypes
import numpy as np

P = 128
HIDDEN = 2048
INTER = 1408
GU = 2 * INTER            # 2816 gate+up columns
KH = HIDDEN // P          # 16 k-tiles for GEMM1
KI = INTER // P           # 11 feature groups / GEMM2 k-tiles
NO = HIDDEN // 512        # 4 output column blocks of 512
N_CORES = 8
NT = 256                  # tokens per chunk
TB = NT // P              # 128-token blocks per chunk (2)

BF16 = ml_dtypes.bfloat16

_PROGRAM_CACHE: dict = {}


# --------------------------------------------------------------------------
# chunk -> slot packing
# --------------------------------------------------------------------------

def _structures(c):
    """All descending partitions of c into 1..4 parts of size <= 6."""
    out = []

    def rec(rem, maxp, cur):
        if rem == 0:
            out.append(tuple(cur))
            return
        if len(cur) == 4:
            return
        for p in range(min(maxp, rem), 0, -1):
            rec(rem - p, p, cur + [p])

    rec(c, min(c, 6), [])
    out.sort(key=lambda s: (len(s), -min(s)))
    return out


def _pack(m, S):
    """Pack expert chunk-counts m into 8 copies of slot structure S.

    Returns {(core, slot_idx): (expert, n_chunks)} or None.  A slot holds
    chunks of a single expert and may be partially filled (padding)."""
    slots = []
    for si, sz in enumerate(S):
        for core in range(N_CORES):
            slots.append((sz, core, si))
    slots.sort(key=lambda t: -t[0])
    rem = list(m)
    assign = {}
    nodes = [0]

    def feasible(i):
        caps = [s[0] for s in slots[i:]]
        need = [r for r in rem if r > 0]
        if not need:
            return True
        if not caps or sum(caps) < sum(need):
            return False
        mx = max(caps)
        return sum((r + mx - 1) // mx for r in need) <= len(caps)

    def rec(i):
        nodes[0] += 1
        if nodes[0] > 300000:
            return False
        if all(r == 0 for r in rem):
            return True
        if i == len(slots) or not feasible(i):
            return False
        sz, core, si = slots[i]
        cands = sorted(
            (e for e in range(len(rem)) if rem[e] > 0),
            key=lambda e: (rem[e] != sz, -rem[e]),
        )
        tried = set()
        for e in cands:
            amt = min(rem[e], sz)
            if amt in tried:
                continue
            tried.add(amt)
            rem[e] -= amt
            assign[(core, si)] = (e, amt)
            if rec(i + 1):
                return True
            del assign[(core, si)]
            rem[e] += amt
        return rec(i + 1)  # leave this slot empty

    return assign if rec(0) else None


def _plan(counts):
    """-> (S, cores) where cores[r] = [(expert|None, echunk0, n_real), ...]
    one entry per slot of S."""
    m = [(c + NT - 1) // NT for c in counts]
    total = sum(m)
    base = (total + N_CORES - 1) // N_CORES
    for c in range(base, base + 9):
        for S in _structures(c):
            asg = _pack(list(m), S)
            if asg is None:
                continue
            # hand out chunk ranges per expert in deterministic slot order
            slots = []
            for si, sz in enumerate(S):
                for core in range(N_CORES):
                    slots.append((sz, core, si))
            slots.sort(key=lambda t: -t[0])
            nxt = [0] * len(m)
            cores = [[None] * len(S) for _ in range(N_CORES)]
            for sz, core, si in slots:
                ent = asg.get((core, si))
                if ent is None:
                    continue
                e, amt = ent
                cores[core][si] = (e, nxt[e], amt)
                nxt[e] += amt
            return S, cores
    raise RuntimeError(f"no packing found for counts {counts}")


# --------------------------------------------------------------------------
# device program
# --------------------------------------------------------------------------

def _build_program(S):
    import concourse.mybir as mybir
    import concourse.tile as tile
    from concourse import bacc

    bf16 = mybir.dt.bfloat16
    f32 = mybir.dt.float32

    n_slots = len(S)
    n_chunks = sum(S)

    nc = bacc.Bacc(None, target_bir_lowering=False, debug=False)
    # x: chunk-major, hidden on partitions; each chunk one contiguous 1 MiB DMA
    xT = nc.dram_tensor("xT", [n_chunks, P, KH, NT], bf16, kind="ExternalInput")
    # w1: per (slot, group): [P, 2(gate/up), KH, P] contiguous 1 MiB blocks
    w1 = nc.dram_tensor(
        "w1", [n_slots, KI, P, 2, KH, P], bf16, kind="ExternalInput"
    )
    # w2: per slot: [P, KI, HIDDEN] contiguous 5.5 MiB block
    w2 = nc.dram_tensor("w2", [n_slots, P, KI, HIDDEN], bf16, kind="ExternalInput")
    out = nc.dram_tensor(
        "out", [n_chunks, TB, NO, P, 512], bf16, kind="ExternalOutput"
    )

    with tile.TileContext(nc) as tc, ExitStack() as ctx:
        w1_pool = ctx.enter_context(tc.tile_pool(name="w1p", bufs=6))
        w2_pool = ctx.enter_context(tc.tile_pool(name="w2p", bufs=1))
        x_pool = ctx.enter_context(tc.tile_pool(name="xp", bufs=6))
        h_pool = ctx.enter_context(tc.tile_pool(name="hp", bufs=6))
        g_pool = ctx.enter_context(tc.tile_pool(name="gp", bufs=3))
        o_pool = ctx.enter_context(tc.tile_pool(name="op", bufs=4))
        ps1 = ctx.enter_context(tc.tile_pool(name="ps1", bufs=2, space="PSUM"))
        ps2 = ctx.enter_context(tc.tile_pool(name="ps2", bufs=3, space="PSUM"))

        g0 = 0
        for si, sz in enumerate(S):
            # ---- DMA emission for this slot ----
            # order on the sync ring: x chunk0, w1 group0, rest of x, then
            # w1 groups 1..10 (first matmul gates on ~2 MiB only).
            # w2 + out stores ride the scalar ring so they never head-of-
            # line block the sync ring.
            xts = []
            w1ts = []
            for j in range(sz):
                t = x_pool.tile([P, KH, NT], bf16, tag="xt")
                nc.sync.dma_start(t[:], xT[g0 + j])
                xts.append(t)
                if j == 0:
                    t0 = w1_pool.tile([P, 2, KH, P], bf16, tag="w1g")
                    nc.sync.dma_start(t0[:], w1[si, 0])
                    w1ts.append(t0)
            for mp in range(1, KI):
                t = w1_pool.tile([P, 2, KH, P], bf16, tag="w1g")
                nc.sync.dma_start(t[:], w1[si, mp])
                w1ts.append(t)
            w2t = w2_pool.tile([P, KI, HIDDEN], bf16, tag="w2t")

            # ---- GEMM1 (group-major over the slot's chunks) ----
            hts = []
            for _ in range(sz):
                ht = h_pool.tile([P, KI, NT], bf16, tag="ht")
                hts.append(ht)
            for mp in range(KI):
                w1t = w1ts[mp]
                for j in range(sz):
                    pg = ps1.tile([P, NT], f32, tag="pg")
                    pu = ps1.tile([P, NT], f32, tag="pu")
                    for k in range(KH):
                        nc.tensor.matmul(
                            pg[:],
                            w1t[:, 0, k],
                            xts[j][:, k],
                            start=(k == 0),
                            stop=(k == KH - 1),
                        )
                    for k in range(KH):
                        nc.tensor.matmul(
                            pu[:],
                            w1t[:, 1, k],
                            xts[j][:, k],
                            start=(k == 0),
                            stop=(k == KH - 1),
                        )
                    gt = g_pool.tile([P, NT], bf16, tag="gt")
                    nc.scalar.activation(
                        gt[:], pg[:], mybir.ActivationFunctionType.Silu
                    )
                    nc.vector.tensor_mul(hts[j][:, mp], gt[:], pu[:])
                if mp == 1:
                    # one big w2 DMA, but held back behind group 1's silu on
                    # the scalar queue: issued up front it front-runs the
                    # critical x/w1 loads (21 us startup stall); split into
                    # 11 small pieces the DMA engines run well below line
                    # rate.  Delayed ~2 group periods it rides spare
                    # bandwidth and still lands long before GEMM2.
                    nc.scalar.dma_start(w2t[:], w2[si])

            # ---- GEMM2 (tokens on PSUM partitions) ----
            for j in range(sz):
                for tb in range(TB):
                    for m in range(NO):
                        po = ps2.tile([P, 512], f32, tag="po")
                        for k in range(KI):
                            nc.tensor.matmul(
                                po[:],
                                hts[j][:, k, tb * P : (tb + 1) * P],
                                w2t[:, k, m * 512 : (m + 1) * 512],
                                start=(k == 0),
                                stop=(k == KI - 1),
                            )
                        om = o_pool.tile([P, 512], bf16, tag="om")
                        nc.vector.tensor_copy(om[:], po[:])
                        nc.scalar.dma_start(out[g0 + j, tb, m], om[:])
            g0 += sz
    nc.compile()
    return nc


def _get_program(S):
    if S not in _PROGRAM_CACHE:
        _PROGRAM_CACHE[S] = _build_program(S)
    return _PROGRAM_CACHE[S]


# --------------------------------------------------------------------------
# host-side pack / unpack
# --------------------------------------------------------------------------

def _pack_w1(w: np.ndarray) -> np.ndarray:
    # [HIDDEN, GU] f32 -> [KI, P, 2, KH, P] bf16  (h = 128k + p, c = g*INTER
    # + 128*mp + j)
    return np.ascontiguousarray(
        w.reshape(KH, P, 2, KI, P).transpose(3, 1, 2, 0, 4)
    ).astype(BF16)


def _pack_w2(w: np.ndarray) -> np.ndarray:
    # [INTER, HIDDEN] f32 -> [P, KI, HIDDEN] bf16
    return np.ascontiguousarray(
        w.reshape(KI, P, HIDDEN).transpose(1, 0, 2)
    ).astype(BF16)


def _run(
    hidden_states: np.ndarray,
    merged_gate_up_proj: np.ndarray,
    merged_down_proj: np.ndarray,
    num_tokens_per_expert: np.ndarray,
    trace: bool = False,
):
    counts = [int(c) for c in np.asarray(num_tokens_per_expert)]
    n_exp = len(counts)
    offs = np.concatenate([[0], np.cumsum(counts)]).astype(int)
    total = int(offs[-1])

    S, cores = _plan(counts)
    n_slots = len(S)
    n_chunks = sum(S)
    slot_base = np.concatenate([[0], np.cumsum(S)]).astype(int)

    nc = _get_program(S)

    from concurrent.futures import ThreadPoolExecutor

    pool = ThreadPoolExecutor(8)

    # [TOTAL, HIDDEN] f32 -> bf16 -> [P, KH, total] transposed view source
    x_bf16 = hidden_states[:total].astype(BF16)
    xT_full = np.empty((HIDDEN, total), dtype=BF16)

    def _tr(k):
        xT_full[k * P : (k + 1) * P] = x_bf16[:, k * P : (k + 1) * P].T

    list(pool.map(_tr, range(KH)))
    xT_pkt = xT_full.reshape(KH, P, total).transpose(1, 0, 2)  # [P, KH, total]

    w1_packed = list(pool.map(
        lambda e: _pack_w1(merged_gate_up_proj[e]), range(n_exp)
    ))
    w2_packed = list(pool.map(
        lambda e: _pack_w2(merged_down_proj[e]), range(n_exp)
    ))

    def _core_inputs(r):
        xc = np.zeros((n_chunks, P, KH, NT), dtype=BF16)
        w1c = np.empty((n_slots, KI, P, 2, KH, P), dtype=BF16)
        w2c = np.empty((n_slots, P, KI, HIDDEN), dtype=BF16)
        for si in range(n_slots):
            ent = cores[r][si]
            e = ent[0] if ent is not None else 0
            w1c[si] = w1_packed[e]
            w2c[si] = w2_packed[e]
            if ent is None:
                continue
            e, k0, amt = ent
            for j in range(amt):
                t0 = (k0 + j) * NT
                n = min(NT, counts[e] - t0)
                if n <= 0:
                    break
                xc[slot_base[si] + j, :, :, :n] = xT_pkt[
                    :, :, offs[e] + t0 : offs[e] + t0 + n
                ]
        return {"xT": xc, "w1": w1c, "w2": w2c}

    in_maps = list(pool.map(_core_inputs, range(N_CORES)))
    pool.shutdown(wait=True)

    res = _execute(nc, in_maps, trace)

    out = np.empty((total, HIDDEN), dtype=np.float32)

    def _unshard(r):
        o = res.results[r]["out"]  # [n_chunks, TB, NO, P, 512] bf16
        o = o.transpose(0, 1, 3, 2, 4).reshape(n_chunks, NT, HIDDEN)
        for si in range(n_slots):
            ent = cores[r][si]
            if ent is None:
                continue
            e, k0, amt = ent
            for j in range(amt):
                t0 = (k0 + j) * NT
                n = min(NT, counts[e] - t0)
                if n <= 0:
                    break
                out[offs[e] + t0 : offs[e] + t0 + n] = o[
                    slot_base[si] + j, :n
                ].astype(np.float32)

    upool = ThreadPoolExecutor(8)
    list(upool.map(_unshard, range(N_CORES)))
    upool.shutdown(wait=True)
    return out, res


# --------------------------------------------------------------------------
# execution (pjrt fast path with on-device zero outputs, axon fallback)
# --------------------------------------------------------------------------

def _execute(nc, in_maps, trace):
    from concourse.bass_utils import run_bass_kernel_spmd

    if not trace:
        try:
            return _execute_pjrt_dev_zeros(nc, in_maps)
        except Exception:
            pass
    return run_bass_kernel_spmd(
        nc, in_maps, list(range(N_CORES)), trace=trace
    )


_EXEC_CACHE: dict = {}


def _build_pjrt_executor(nc):
    from concourse.bass_utils import axon_active
    import concourse.mybir as mybir
    from concourse import bass2jax
    import jax
    import jax.numpy as jnp
    from jax.sharding import Mesh, PartitionSpec, NamedSharding
    from jax.experimental.shard_map import shard_map

    if not axon_active():
        raise RuntimeError("pjrt path requires axon")
    if nc.dbg_addr is not None:
        raise RuntimeError("debug program")

    bass2jax.install_neuronx_cc_hook()

    partition_name = nc.partition_id_tensor.name if nc.partition_id_tensor else None
    in_names, out_names, out_avals = [], [], []
    for alloc in nc.m.functions[0].allocations:
        if not isinstance(alloc, mybir.MemoryLocationSet):
            continue
        name = alloc.memorylocations[0].name
        if alloc.kind == "ExternalInput":
            if name != partition_name:
                in_names.append(name)
        elif alloc.kind == "ExternalOutput":
            out_names.append(name)
            out_avals.append(
                jax.core.ShapedArray(
                    tuple(alloc.tensor_shape), mybir.dt.np(alloc.dtype)
                )
            )
    n_params = len(in_names)
    n_outs = len(out_avals)
    all_names = in_names + out_names
    if partition_name is not None:
        all_names = all_names + [partition_name]
    donate = tuple(range(n_params, n_params + n_outs))

    def _body(*args):
        operands = list(args)
        if partition_name is not None:
            operands.append(bass2jax.partition_id_tensor())
        outs = bass2jax._bass_exec_p.bind(
            *operands,
            out_avals=tuple(out_avals),
            in_names=tuple(all_names),
            out_names=tuple(out_names),
            lowering_input_output_aliases=(),
            sim_require_finite=True,
            sim_require_nnan=True,
            nc=nc,
        )
        return tuple(outs)

    devices = jax.devices()[:N_CORES]
    assert len(devices) == N_CORES
    mesh = Mesh(np.asarray(devices), ("core",))
    in_specs = (PartitionSpec("core"),) * (n_params + n_outs)
    out_specs = (PartitionSpec("core"),) * n_outs
    sharded = jax.jit(
        shard_map(
            _body, mesh=mesh, in_specs=in_specs, out_specs=out_specs,
            check_rep=False,
        ),
        donate_argnums=donate,
        keep_unused=True,
    )
    zsharding = NamedSharding(mesh, PartitionSpec("core"))
    zero_fns = [
        jax.jit(
            lambda s=av.shape, d=av.dtype: jnp.zeros(
                (N_CORES * s[0], *s[1:]), d
            ),
            out_shardings=zsharding,
        )
        for av in out_avals
    ]
    return {
        "sharded": sharded,
        "zero_fns": zero_fns,
        "in_names": in_names,
        "out_names": out_names,
        "out_avals": out_avals,
    }


def _execute_pjrt_dev_zeros(nc, in_maps):
    """run_bass_via_pjrt equivalent with donated zero output buffers created
    on-device instead of staged from host numpy."""
    from concourse.bass_utils import BassKernelResults

    key = id(nc)
    if key not in _EXEC_CACHE:
        _EXEC_CACHE[key] = _build_pjrt_executor(nc)
    ex = _EXEC_CACHE[key]

    concat_in = [
        np.concatenate([np.asarray(m[name]) for m in in_maps], axis=0)
        for name in ex["in_names"]
    ]
    dev_zeros = [fn() for fn in ex["zero_fns"]]
    out_arrs = ex["sharded"](*concat_in, *dev_zeros)
    out_avals = ex["out_avals"]
    results = [
        {
            name: np.asarray(out_arrs[i]).reshape(
                N_CORES, *out_avals[i].shape
            )[c]
            for i, name in enumerate(ex["out_names"])
        }
        for c in range(N_CORES)
    ]
    return BassKernelResults(
        results=results,
        instructions_and_trace=None,
        profile_json=None,
        exec_time_ns=None,
    )


def kernel(**inputs) -> np.ndarray:
    return _run(**inputs, trace=False)[0]


def run_traced(**inputs):
    return _run(**inputs, trace=True)


# revision 8
# speedup vs baseline: 1.0321x; 1.0285x over previous
"""Grouped MoE MLP (SwiGLU) kernel for Trainium2, 8 NeuronCores.

Strategy (load-balanced expert-parallel):
  The per-expert token counts are ragged (max 3072 vs mean 2048), so the
  baseline one-expert-per-core split leaves the hot core with 1.5x the
  average work -- and the trace shows TensorE is 96% busy, i.e. the
  kernel is at the matmul roofline for whatever token count the hot
  core carries.  The only lever is balance.

  Tokens are split into NT=256-token chunks (every chunk belongs to one
  expert; tokens arrive pre-sorted by expert).  The 64 chunks are packed
  into 8 cores x fixed per-core "slots" (e.g. sizes (3,3,2) chunks): one
  slot processes chunks of a single expert, so each core runs exactly
  sum(S) chunks = 2048 tokens.  A small backtracking packer finds a
  zero-waste structure for the given counts (for the reference counts
  the (3,3,2) packing is exact).

  Per-slot weights are streamed from HBM: GEMM1 weights at (gate,up)
  column-group granularity through a small SBUF ring (a group is dead
  once its GEMM1 finishes, so the full 11.5 MiB never sits in SBUF),
  GEMM2 weights one 5.8 MiB block per slot, double-use-free.  DMA per
  core totals ~69 MiB against ~190 us of bandwidth -- well hidden under
  ~460 us of matmul work.

  Device program per slot (dense SwiGLU over the slot's chunks):
    GEMM1 group-major: for mp in 0..10: stream w1[mp] (gate+up), then
      for each chunk: accumulate 16 k-tiles into PSUM for gate and up,
      SiLU (ACT) * up (DVE) -> h[:, mp, :] in SBUF (bf16)
    GEMM2 per chunk: tokens on the PSUM partition dim, out lands in
      natural [T, HIDDEN] layout.

  All device I/O is bf16 (cast on host) to halve staged bytes.
"""

import math
from contextlib import ExitStack

import ml_dtypes
import numpy as np

P = 128
HIDDEN = 2048
INTER = 1408
GU = 2 * INTER            # 2816 gate+up columns
KH = HIDDEN // P          # 16 k-tiles for GEMM1
KI = INTER // P           # 11 feature groups / GEMM2 k-tiles
NO = HIDDEN // 512        # 4 output column blocks of 512
N_CORES = 8
NT = 256                  # tokens per chunk
TB = NT // P              # 128-token blocks per chunk (2)

BF16 = ml_dtypes.bfloat16

_PROGRAM_CACHE: dict = {}


# --------------------------------------------------------------------------
# chunk -> slot packing
# --------------------------------------------------------------------------

def _structures(c):
    """All descending partitions of c into 1..4 parts of size <= 6."""
    out = []

    def rec(rem, maxp, cur):
        if rem == 0:
            out.append(tuple(cur))
            return
        if len(cur) == 4:
            return
        for p in range(min(maxp, rem), 0, -1):
            rec(rem - p, p, cur + [p])

    rec(c, min(c, 6), [])
    out.sort(key=lambda s: (len(s), -min(s)))
    return out


def _pack(m, S):
    """Pack expert chunk-counts m into 8 copies of slot structure S.

    Returns {(core, slot_idx): (expert, n_chunks)} or None.  A slot holds
    chunks of a single expert and may be partially filled (padding)."""
    slots = []
    for si, sz in enumerate(S):
        for core in range(N_CORES):
            slots.append((sz, core, si))
    slots.sort(key=lambda t: -t[0])
    rem = list(m)
    assign = {}
    nodes = [0]

    def feasible(i):
        caps = [s[0] for s in slots[i:]]
        need = [r for r in rem if r > 0]
        if not need:
            return True
        if not caps or sum(caps) < sum(need):
            return False
        mx = max(caps)
        return sum((r + mx - 1) // mx for r in need) <= len(caps)

    def rec(i):
        nodes[0] += 1
        if nodes[0] > 300000:
            return False
        if all(r == 0 for r in rem):
            return True
        if i == len(slots) or not feasible(i):
            return False
        sz, core, si = slots[i]
        cands = sorted(
            (e for e in range(len(rem)) if rem[e] > 0),
            key=lambda e: (rem[e] != sz, -rem[e]),
        )
        tried = set()
        for e in cands:
            amt = min(rem[e], sz)
            if amt in tried:
                continue
            tried.add(amt)
            rem[e] -= amt
            assign[(core, si)] = (e, amt)
            if rec(i + 1):
                return True
            del assign[(core, si)]
            rem[e] += amt
        return rec(i + 1)  # leave this slot empty

    return assign if rec(0) else None


def _plan(counts):
    """-> (S, cores) where cores[r] = [(expert|None, echunk0, n_real), ...]
    one entry per slot of S."""
    m = [(c + NT - 1) // NT for c in counts]
    total = sum(m)
    base = (total + N_CORES - 1) // N_CORES
    for c in range(base, base + 9):
        for S in _structures(c):
            asg = _pack(list(m), S)
            if asg is None:
                continue
            # hand out chunk ranges per expert in deterministic slot order
            slots = []
            for si, sz in enumerate(S):
                for core in range(N_CORES):
                    slots.append((sz, core, si))
            slots.sort(key=lambda t: -t[0])
            nxt = [0] * len(m)
            cores = [[None] * len(S) for _ in range(N_CORES)]
            for sz, core, si in slots:
                ent = asg.get((core, si))
                if ent is None:
                    continue
                e, amt = ent
                cores[core][si] = (e, nxt[e], amt)
                nxt[e] += amt
            return S, cores
    raise RuntimeError(f"no packing found for counts {counts}")


# --------------------------------------------------------------------------
# device program
# --------------------------------------------------------------------------

def _build_program(S):
    import concourse.mybir as mybir
    import concourse.tile as tile
    from concourse import bacc

    bf16 = mybir.dt.bfloat16
    f32 = mybir.dt.float32

    n_slots = len(S)
    n_chunks = sum(S)

    nc = bacc.Bacc(None, target_bir_lowering=False, debug=False)
    # x: chunk-major, hidden on partitions; each chunk one contiguous 1 MiB DMA
    xT = nc.dram_tensor("xT", [n_chunks, P, KH, NT], bf16, kind="ExternalInput")
    # w1: per (slot, group): [P, 2(gate/up), KH, P] contiguous 1 MiB blocks
    w1 = nc.dram_tensor(
        "w1", [n_slots, KI, P, 2, KH, P], bf16, kind="ExternalInput"
    )
    # w2: per slot: [P, KI, HIDDEN] contiguous 5.5 MiB block
    w2 = nc.dram_tensor("w2", [n_slots, P, KI, HIDDEN], bf16, kind="ExternalInput")
    out = nc.dram_tensor(
        "out", [n_chunks, TB, NO, P, 512], bf16, kind="ExternalOutput"
    )

    with tile.TileContext(nc) as tc, ExitStack() as ctx:
        w1_pool = ctx.enter_context(tc.tile_pool(name="w1p", bufs=6))
        w2_pool = ctx.enter_context(tc.tile_pool(name="w2p", bufs=1))
        x_pool = ctx.enter_context(tc.tile_pool(name="xp", bufs=6))
        h_pool = ctx.enter_context(tc.tile_pool(name="hp", bufs=6))
        g_pool = ctx.enter_context(tc.tile_pool(name="gp", bufs=3))
        o_pool = ctx.enter_context(tc.tile_pool(name="op", bufs=4))
        ps1 = ctx.enter_context(tc.tile_pool(name="ps1", bufs=2, space="PSUM"))
        ps2 = ctx.enter_context(tc.tile_pool(name="ps2", bufs=3, space="PSUM"))

        g0 = 0
        for si, sz in enumerate(S):
            # ---- DMA emission for this slot ----
            # order on the sync ring: x chunk0, w1 group0, rest of x, then
            # w1 groups 1..10 (first matmul gates on ~2 MiB only).
            # w2 + out stores ride the scalar ring so they never head-of-
            # line block the sync ring.
            xts = []
            w1ts = []
            for j in range(sz):
                t = x_pool.tile([P, KH, NT], bf16, tag="xt")
                nc.sync.dma_start(t[:], xT[g0 + j])
                xts.append(t)
                if j == 0:
                    t0 = w1_pool.tile([P, 2, KH, P], bf16, tag="w1g")
                    nc.sync.dma_start(t0[:], w1[si, 0])
                    w1ts.append(t0)
            for mp in range(1, KI):
                t = w1_pool.tile([P, 2, KH, P], bf16, tag="w1g")
                nc.sync.dma_start(t[:], w1[si, mp])
                w1ts.append(t)
            w2t = w2_pool.tile([P, KI, HIDDEN], bf16, tag="w2t")

            # ---- GEMM1 (group-major over the slot's chunks) ----
            hts = []
            for _ in range(sz):
                ht = h_pool.tile([P, KI, NT], bf16, tag="ht")
                hts.append(ht)
            for mp in range(KI):
                w1t = w1ts[mp]
                for j in range(sz):
                    pg = ps1.tile([P, NT], f32, tag="pg")
                    pu = ps1.tile([P, NT], f32, tag="pu")
                    for k in range(KH):
                        nc.tensor.matmul(
                            pg[:],
                            w1t[:, 0, k],
                            xts[j][:, k],
                            start=(k == 0),
                            stop=(k == KH - 1),
                        )
                    for k in range(KH):
                        nc.tensor.matmul(
                            pu[:],
                            w1t[:, 1, k],
                            xts[j][:, k],
                            start=(k == 0),
                            stop=(k == KH - 1),
                        )
                    gt = g_pool.tile([P, NT], bf16, tag="gt")
                    nc.scalar.activation(
                        gt[:], pg[:], mybir.ActivationFunctionType.Silu
                    )
                    nc.vector.tensor_mul(hts[j][:, mp], gt[:], pu[:])
                # w2 rides in KI ~0.5 MiB pieces: one monolithic 5.8 MiB DMA
                # head-of-line blocks the critical x/w1 stream at the SDMA
                # engine level (engines drain a whole descriptor batch per
                # queue before switching -- measured: x0's completion
                # semaphore fired at 29 us instead of ~13 us).  Slot 0's
                # pieces additionally carry a scheduler-time floor
                # (tile_wait_until) so the priority heap cannot hoist them
                # into the startup window; emission order alone is NOT
                # preserved by the Tile scheduler for dep-free DMAs.
                if si == 0:
                    with tc.tile_wait_until(0.025 + 0.005 * mp):
                        nc.scalar.dma_start(w2t[:, mp], w2[si, :, mp])
                else:
                    nc.scalar.dma_start(w2t[:, mp], w2[si, :, mp])

            # ---- GEMM2 (tokens on PSUM partitions) ----
            for j in range(sz):
                for tb in range(TB):
                    for m in range(NO):
                        po = ps2.tile([P, 512], f32, tag="po")
                        for k in range(KI):
                            nc.tensor.matmul(
                                po[:],
                                hts[j][:, k, tb * P : (tb + 1) * P],
                                w2t[:, k, m * 512 : (m + 1) * 512],
                                start=(k == 0),
                                stop=(k == KI - 1),
                            )
                        om = o_pool.tile([P, 512], bf16, tag="om")
                        nc.vector.tensor_copy(om[:], po[:])
                        nc.scalar.dma_start(out[g0 + j, tb, m], om[:])
            g0 += sz
    nc.compile()
    return nc


def _get_program(S):
    if S not in _PROGRAM_CACHE:
        _PROGRAM_CACHE[S] = _build_program(S)
    return _PROGRAM_CACHE[S]


# --------------------------------------------------------------------------
# host-side pack / unpack
# --------------------------------------------------------------------------

def _pack_w1(w: np.ndarray) -> np.ndarray:
    # [HIDDEN, GU] f32 -> [KI, P, 2, KH, P] bf16  (h = 128k + p, c = g*INTER
    # + 128*mp + j)
    return np.ascontiguousarray(
        w.reshape(KH, P, 2, KI, P).transpose(3, 1, 2, 0, 4)
    ).astype(BF16)


def _pack_w2(w: np.ndarray) -> np.ndarray:
    # [INTER, HIDDEN] f32 -> [P, KI, HIDDEN] bf16
    return np.ascontiguousarray(
        w.reshape(KI, P, HIDDEN).transpose(1, 0, 2)
    ).astype(BF16)


def _run(
    hidden_states: np.ndarray,
    merged_gate_up_proj: np.ndarray,
    merged_down_proj: np.ndarray,
    num_tokens_per_expert: np.ndarray,
    trace: bool = False,
):
    counts = [int(c) for c in np.asarray(num_tokens_per_expert)]
    n_exp = len(counts)
    offs = np.concatenate([[0], np.cumsum(counts)]).astype(int)
    total = int(offs[-1])

    S, cores = _plan(counts)
    n_slots = len(S)
    n_chunks = sum(S)
    slot_base = np.concatenate([[0], np.cumsum(S)]).astype(int)

    nc = _get_program(S)

    from concurrent.futures import ThreadPoolExecutor

    pool = ThreadPoolExecutor(8)

    # [TOTAL, HIDDEN] f32 -> bf16 -> [P, KH, total] transposed view source
    x_bf16 = hidden_states[:total].astype(BF16)
    xT_full = np.empty((HIDDEN, total), dtype=BF16)

    def _tr(k):
        xT_full[k * P : (k + 1) * P] = x_bf16[:, k * P : (k + 1) * P].T

    list(pool.map(_tr, range(KH)))
    xT_pkt = xT_full.reshape(KH, P, total).transpose(1, 0, 2)  # [P, KH, total]

    w1_packed = list(pool.map(
        lambda e: _pack_w1(merged_gate_up_proj[e]), range(n_exp)
    ))
    w2_packed = list(pool.map(
        lambda e: _pack_w2(merged_down_proj[e]), range(n_exp)
    ))

    def _core_inputs(r):
        xc = np.zeros((n_chunks, P, KH, NT), dtype=BF16)
        w1c = np.empty((n_slots, KI, P, 2, KH, P), dtype=BF16)
        w2c = np.empty((n_slots, P, KI, HIDDEN), dtype=BF16)
        for si in range(n_slots):
            ent = cores[r][si]
            e = ent[0] if ent is not None else 0
            w1c[si] = w1_packed[e]
            w2c[si] = w2_packed[e]
            if ent is None:
                continue
            e, k0, amt = ent
            for j in range(amt):
                t0 = (k0 + j) * NT
                n = min(NT, counts[e] - t0)
                if n <= 0:
                    break
                xc[slot_base[si] + j, :, :, :n] = xT_pkt[
                    :, :, offs[e] + t0 : offs[e] + t0 + n
                ]
        return {"xT": xc, "w1": w1c, "w2": w2c}

    in_maps = list(pool.map(_core_inputs, range(N_CORES)))
    pool.shutdown(wait=True)

    res = _execute(nc, in_maps, trace)

    out = np.empty((total, HIDDEN), dtype=np.float32)

    def _unshard(r):
        o = res.results[r]["out"]  # [n_chunks, TB, NO, P, 512] bf16
        o = o.transpose(0, 1, 3, 2, 4).reshape(n_chunks, NT, HIDDEN)
        for si in range(n_slots):
            ent = cores[r][si]
            if ent is None:
                continue
            e, k0, amt = ent
            for j in range(amt):
                t0 = (k0 + j) * NT
                n = min(NT, counts[e] - t0)
                if n <= 0:
                    break
                out[offs[e] + t0 : offs[e] + t0 + n] = o[
                    slot_base[si] + j, :n
                ].astype(np.float32)

    upool = ThreadPoolExecutor(8)
    list(upool.map(_unshard, range(N_CORES)))
    upool.shutdown(wait=True)
    return out, res


# --------------------------------------------------------------------------
# execution (pjrt fast path with on-device zero outputs, axon fallback)
# --------------------------------------------------------------------------

def _execute(nc, in_maps, trace):
    from concourse.bass_utils import run_bass_kernel_spmd

    if not trace:
        try:
            return _execute_pjrt_dev_zeros(nc, in_maps)
        except Exception:
            pass
    return run_bass_kernel_spmd(
        nc, in_maps, list(range(N_CORES)), trace=trace
    )


_EXEC_CACHE: dict = {}


def _build_pjrt_executor(nc):
    from concourse.bass_utils import axon_active
    import concourse.mybir as mybir
    from concourse import bass2jax
    import jax
    import jax.numpy as jnp
    from jax.sharding import Mesh, PartitionSpec, NamedSharding
    from jax.experimental.shard_map import shard_map

    if not axon_active():
        raise RuntimeError("pjrt path requires axon")
    if nc.dbg_addr is not None:
        raise RuntimeError("debug program")

    bass2jax.install_neuronx_cc_hook()

    partition_name = nc.partition_id_tensor.name if nc.partition_id_tensor else None
    in_names, out_names, out_avals = [], [], []
    for alloc in nc.m.functions[0].allocations:
        if not isinstance(alloc, mybir.MemoryLocationSet):
            continue
        name = alloc.memorylocations[0].name
        if alloc.kind == "ExternalInput":
            if name != partition_name:
                in_names.append(name)
        elif alloc.kind == "ExternalOutput":
            out_names.append(name)
            out_avals.append(
                jax.core.ShapedArray(
                    tuple(alloc.tensor_shape), mybir.dt.np(alloc.dtype)
                )
            )
    n_params = len(in_names)
    n_outs = len(out_avals)
    all_names = in_names + out_names
    if partition_name is not None:
        all_names = all_names + [partition_name]
    donate = tuple(range(n_params, n_params + n_outs))

    def _body(*args):
        operands = list(args)
        if partition_name is not None:
            operands.append(bass2jax.partition_id_tensor())
        outs = bass2jax._bass_exec_p.bind(
            *operands,
            out_avals=tuple(out_avals),
            in_names=tuple(all_names),
            out_names=tuple(out_names),
            lowering_input_output_aliases=(),
            sim_require_finite=True,
            sim_require_nnan=True,
            nc=nc,
        )
        return tuple(outs)

    devices = jax.devices()[:N_CORES]
    assert len(devices) == N_CORES
    mesh = Mesh(np.asarray(devices), ("core",))
    in_specs = (PartitionSpec("core"),) * (n_params + n_outs)
    out_specs = (PartitionSpec("core"),) * n_outs
    sharded = jax.jit(
        shard_map(
            _body, mesh=mesh, in_specs=in_specs, out_specs=out_specs,
            check_rep=False,
        ),
        donate_argnums=donate,
        keep_unused=True,
    )
    zsharding = NamedSharding(mesh, PartitionSpec("core"))
    zero_fns = [
        jax.jit(
            lambda s=av.shape, d=av.dtype: jnp.zeros(
                (N_CORES * s[0], *s[1:]), d
            ),
            out_shardings=zsharding,
        )
        for av in out_avals
    ]
    return {
        "sharded": sharded,
        "zero_fns": zero_fns,
        "in_names": in_names,
        "out_names": out_names,
        "out_avals": out_avals,
    }


def _execute_pjrt_dev_zeros(nc, in_maps):
    """run_bass_via_pjrt equivalent with donated zero output buffers created
    on-device instead of staged from host numpy."""
    from concourse.bass_utils import BassKernelResults

    key = id(nc)
    if key not in _EXEC_CACHE:
        _EXEC_CACHE[key] = _build_pjrt_executor(nc)
    ex = _EXEC_CACHE[key]

    concat_in = [
        np.concatenate([np.asarray(m[name]) for m in in_maps], axis=0)
        for name in ex["in_names"]
    ]
    dev_zeros = [fn() for fn in ex["zero_fns"]]
    out_arrs = ex["sharded"](*concat_in, *dev_zeros)
    out_avals = ex["out_avals"]
    results = [
        {
            name: np.asarray(out_arrs[i]).reshape(
                N_CORES, *out_avals[i].shape
            )[c]
            for i, name in enumerate(ex["out_names"])
        }
        for c in range(N_CORES)
    ]
    return BassKernelResults(
        results=results,
        instructions_and_trace=None,
        profile_json=None,
        exec_time_ns=None,
    )


def kernel(**inputs) -> np.ndarray:
    return _run(**inputs, trace=False)[0]


def run_traced(**inputs):
    return _run(**inputs, trace=True)


# revision 11
# speedup vs baseline: 1.0409x; 1.0085x over previous
"""Grouped MoE MLP (SwiGLU) kernel for Trainium2, 8 NeuronCores.

Strategy (load-balanced expert-parallel):
  The per-expert token counts are ragged (max 3072 vs mean 2048), so the
  baseline one-expert-per-core split leaves the hot core with 1.5x the
  average work -- and the trace shows TensorE is 96% busy, i.e. the
  kernel is at the matmul roofline for whatever token count the hot
  core carries.  The only lever is balance.

  Tokens are split into NT=256-token chunks (every chunk belongs to one
  expert; tokens arrive pre-sorted by expert).  The 64 chunks are packed
  into 8 cores x fixed per-core "slots" (e.g. sizes (3,3,2) chunks): one
  slot processes chunks of a single expert, so each core runs exactly
  sum(S) chunks = 2048 tokens.  A small backtracking packer finds a
  zero-waste structure for the given counts (for the reference counts
  the (3,3,2) packing is exact).

  Per-slot weights are streamed from HBM: GEMM1 weights at (gate,up)
  column-group granularity through a small SBUF ring (a group is dead
  once its GEMM1 finishes, so the full 11.5 MiB never sits in SBUF),
  GEMM2 weights one 5.8 MiB block per slot, double-use-free.  DMA per
  core totals ~69 MiB against ~190 us of bandwidth -- well hidden under
  ~460 us of matmul work.

  Device program per slot (dense SwiGLU over the slot's chunks):
    GEMM1 group-major: for mp in 0..10: stream w1[mp] (gate+up), then
      for each chunk: accumulate 16 k-tiles into PSUM for gate and up,
      SiLU (ACT) * up (DVE) -> h[:, mp, :] in SBUF (bf16)
    GEMM2 per chunk: tokens on the PSUM partition dim, out lands in
      natural [T, HIDDEN] layout.

  All device I/O is bf16 (cast on host) to halve staged bytes.
"""

import math
from contextlib import ExitStack

import ml_dtypes
import numpy as np

P = 128
HIDDEN = 2048
INTER = 1408
GU = 2 * INTER            # 2816 gate+up columns
KH = HIDDEN // P          # 16 k-tiles for GEMM1
KI = INTER // P           # 11 feature groups / GEMM2 k-tiles
NO = HIDDEN // 512        # 4 output column blocks of 512
N_CORES = 8
NT = 256                  # tokens per chunk
TB = NT // P              # 128-token blocks per chunk (2)

BF16 = ml_dtypes.bfloat16

_PROGRAM_CACHE: dict = {}


# --------------------------------------------------------------------------
# chunk -> slot packing
# --------------------------------------------------------------------------

def _structures(c):
    """All descending partitions of c into 1..4 parts of size <= 6."""
    out = []

    def rec(rem, maxp, cur):
        if rem == 0:
            out.append(tuple(cur))
            return
        if len(cur) == 4:
            return
        for p in range(min(maxp, rem), 0, -1):
            rec(rem - p, p, cur + [p])

    rec(c, min(c, 6), [])
    out.sort(key=lambda s: (len(s), -min(s)))
    return out


def _pack(m, S):
    """Pack expert chunk-counts m into 8 copies of slot structure S.

    Returns {(core, slot_idx): (expert, n_chunks)} or None.  A slot holds
    chunks of a single expert and may be partially filled (padding)."""
    slots = []
    for si, sz in enumerate(S):
        for core in range(N_CORES):
            slots.append((sz, core, si))
    slots.sort(key=lambda t: -t[0])
    rem = list(m)
    assign = {}
    nodes = [0]

    def feasible(i):
        caps = [s[0] for s in slots[i:]]
        need = [r for r in rem if r > 0]
        if not need:
            return True
        if not caps or sum(caps) < sum(need):
            return False
        mx = max(caps)
        return sum((r + mx - 1) // mx for r in need) <= len(caps)

    def rec(i):
        nodes[0] += 1
        if nodes[0] > 300000:
            return False
        if all(r == 0 for r in rem):
            return True
        if i == len(slots) or not feasible(i):
            return False
        sz, core, si = slots[i]
        cands = sorted(
            (e for e in range(len(rem)) if rem[e] > 0),
            key=lambda e: (rem[e] != sz, -rem[e]),
        )
        tried = set()
        for e in cands:
            amt = min(rem[e], sz)
            if amt in tried:
                continue
            tried.add(amt)
            rem[e] -= amt
            assign[(core, si)] = (e, amt)
            if rec(i + 1):
                return True
            del assign[(core, si)]
            rem[e] += amt
        return rec(i + 1)  # leave this slot empty

    return assign if rec(0) else None


def _plan(counts):
    """-> (S, cores) where cores[r] = [(expert|None, echunk0, n_real), ...]
    one entry per slot of S."""
    m = [(c + NT - 1) // NT for c in counts]
    total = sum(m)
    base = (total + N_CORES - 1) // N_CORES
    for c in range(base, base + 9):
        for S in _structures(c):
            asg = _pack(list(m), S)
            if asg is None:
                continue
            # hand out chunk ranges per expert in deterministic slot order
            slots = []
            for si, sz in enumerate(S):
                for core in range(N_CORES):
                    slots.append((sz, core, si))
            slots.sort(key=lambda t: -t[0])
            nxt = [0] * len(m)
            cores = [[None] * len(S) for _ in range(N_CORES)]
            for sz, core, si in slots:
                ent = asg.get((core, si))
                if ent is None:
                    continue
                e, amt = ent
                cores[core][si] = (e, nxt[e], amt)
                nxt[e] += amt
            return S, cores
    raise RuntimeError(f"no packing found for counts {counts}")


# --------------------------------------------------------------------------
# device program
# --------------------------------------------------------------------------

def _build_program(S):
    import concourse.mybir as mybir
    import concourse.tile as tile
    from concourse import bacc

    bf16 = mybir.dt.bfloat16
    f32 = mybir.dt.float32

    n_slots = len(S)
    n_chunks = sum(S)

    nc = bacc.Bacc(None, target_bir_lowering=False, debug=False)
    # x: chunk-major, hidden on partitions; each chunk one contiguous 1 MiB DMA
    xT = nc.dram_tensor("xT", [n_chunks, P, KH, NT], bf16, kind="ExternalInput")
    # w1: per (slot, group): [P, 2(gate/up), KH, P] contiguous 1 MiB blocks
    w1 = nc.dram_tensor(
        "w1", [n_slots, KI, P, 2, KH, P], bf16, kind="ExternalInput"
    )
    # w2: per slot: [P, KI, HIDDEN] contiguous 5.5 MiB block
    w2 = nc.dram_tensor("w2", [n_slots, P, KI, HIDDEN], bf16, kind="ExternalInput")
    out = nc.dram_tensor(
        "out", [n_chunks, TB, NO, P, 512], bf16, kind="ExternalOutput"
    )

    # slot si > 0 is "paired" (slot-contiguous x, N=512 segments) when its
    # token span is small enough to keep the SBUF budget; slot 0 always runs
    # per-chunk so the first matmul gates on a single chunk + w1 group.
    paired = [si > 0 and S[si] * NT <= 768 for si in range(n_slots)]
    pc_szs = sorted(S[si] for si in range(n_slots) if not paired[si])
    x_bufs = pc_szs[-1] + (1 if len(pc_szs) > 1 else 0)
    h_bufs = sum(pc_szs[-2:])
    tok_p = max([S[si] * NT for si in range(n_slots) if paired[si]], default=0)
    kb = (45 + 8 * x_bufs + 5.5 * h_bufs
          + (KH * tok_p * 2 + 2 * KI * tok_p * 2) / 1024 + 14)
    w1_bufs = next(b for b in (5, 4, 3, 2) if kb + 8 * b <= 202)

    with tile.TileContext(nc) as tc, ExitStack() as ctx:
        w1_pool = ctx.enter_context(tc.tile_pool(name="w1p", bufs=w1_bufs))
        w2_pool = ctx.enter_context(tc.tile_pool(name="w2p", bufs=1))
        x_pool = ctx.enter_context(tc.tile_pool(name="xp", bufs=x_bufs))
        xs_pool = ctx.enter_context(tc.tile_pool(name="xsp", bufs=1))
        h_pool = ctx.enter_context(tc.tile_pool(name="hp", bufs=h_bufs))
        hs_pool = ctx.enter_context(tc.tile_pool(name="hsp", bufs=2))
        g_pool = ctx.enter_context(tc.tile_pool(name="gp", bufs=3))
        o_pool = ctx.enter_context(tc.tile_pool(name="op", bufs=4))
        ps1 = ctx.enter_context(tc.tile_pool(name="ps1", bufs=2, space="PSUM"))
        ps2 = ctx.enter_context(tc.tile_pool(name="ps2", bufs=3, space="PSUM"))
        psw = ctx.enter_context(tc.tile_pool(name="psw", bufs=1, space="PSUM"))

        # ---- PE warm-up ----
        # The HAM clock gate keeps the PE at 1.2 GHz until ~3.4 us of
        # sustained matmul activity.  The first real matmul can't start
        # until ~15 us (NEFF preamble + 2.1 MiB of x/w1), so burn dummy
        # matmuls on a zeroed tile during that window: they retire before
        # the data lands and the real stream starts at the warm 2.4 GHz.
        wsrc = g_pool.tile([P, 512], bf16, tag="wsrc")
        nc.vector.memset(wsrc[:], 0)
        pw = psw.tile([P, 256], f32, tag="pw")
        for _ in range(30):
            nc.tensor.matmul(
                pw[:], wsrc[:, 0:P], wsrc[:, 256:512], start=True, stop=True
            )

        g0 = 0
        for si, sz in enumerate(S):
            tok = sz * NT
            # ---- DMA emission for this slot ----
            # order on the sync ring: x chunk0, w1 group0, rest of x, then
            # w1 groups 1..10 (first matmul gates on ~2 MiB only).
            # w2 + out stores ride the scalar ring so they never head-of-
            # line block the sync ring.
            w1ts = []
            if not paired[si]:
                # per-chunk x tiles + 256-wide segments: the first matmul
                # chain gates on one chunk + one w1 group only
                xts = []
                for j in range(sz):
                    t = x_pool.tile([P, KH, NT], bf16, tag="xt")
                    nc.sync.dma_start(t[:], xT[g0 + j])
                    xts.append(t)
                    if j == 0:
                        t0 = w1_pool.tile([P, 2, KH, P], bf16, tag="w1g")
                        nc.sync.dma_start(t0[:], w1[si, 0])
                        w1ts.append(t0)

                def xseg(k, a, n, _xts=xts):
                    return _xts[a // NT][:, k, a % NT : a % NT + n]

                segs = [(j * NT, NT) for j in range(sz)]
            else:
                # slot-contiguous x tile: chunks pair into N=512 matmuls
                # (the per-MM issue overhead is halved; ~1 us per slot)
                xsl = xs_pool.tile([P, KH, tok], bf16, tag="xs")
                for j in range(sz):
                    nc.sync.dma_start(
                        xsl[:, :, j * NT : (j + 1) * NT], xT[g0 + j]
                    )
                    if j == 0:
                        t0 = w1_pool.tile([P, 2, KH, P], bf16, tag="w1g")
                        nc.sync.dma_start(t0[:], w1[si, 0])
                        w1ts.append(t0)

                def xseg(k, a, n, _xsl=xsl):
                    return _xsl[:, k, a : a + n]

                segs = []
                a = 0
                while a < tok:
                    n = min(512, tok - a)
                    segs.append((a, n))
                    a += n
            for mp in range(1, KI):
                t = w1_pool.tile([P, 2, KH, P], bf16, tag="w1g")
                nc.sync.dma_start(t[:], w1[si, mp])
                w1ts.append(t)
            w2t = w2_pool.tile([P, KI, HIDDEN], bf16, tag="w2t")

            # ---- GEMM1 (group-major over the slot's segments) ----
            if not paired[si]:
                hts = []
                for _ in range(sz):
                    ht = h_pool.tile([P, KI, NT], bf16, tag="ht")
                    hts.append(ht)

                def hseg(mp, a, n, _hts=hts):
                    return _hts[a // NT][:, mp, a % NT : a % NT + n]

                def hblk(k, tb, _hts=hts):
                    a = tb * P
                    return _hts[a // NT][:, k, a % NT : a % NT + P]
            else:
                hsl = hs_pool.tile([P, KI, tok], bf16, tag="hs")

                def hseg(mp, a, n, _hsl=hsl):
                    return _hsl[:, mp, a : a + n]

                def hblk(k, tb, _hsl=hsl):
                    return _hsl[:, k, tb * P : (tb + 1) * P]

            for mp in range(KI):
                w1t = w1ts[mp]
                for (a, n) in segs:
                    pg = ps1.tile([P, 512], f32, tag="pg")
                    pu = ps1.tile([P, 512], f32, tag="pu")
                    for k in range(KH):
                        nc.tensor.matmul(
                            pg[:, :n],
                            w1t[:, 0, k],
                            xseg(k, a, n),
                            start=(k == 0),
                            stop=(k == KH - 1),
                        )
                    for k in range(KH):
                        nc.tensor.matmul(
                            pu[:, :n],
                            w1t[:, 1, k],
                            xseg(k, a, n),
                            start=(k == 0),
                            stop=(k == KH - 1),
                        )
                    gt = g_pool.tile([P, 512], bf16, tag="gt")
                    nc.scalar.activation(
                        gt[:, :n], pg[:, :n], mybir.ActivationFunctionType.Silu
                    )
                    nc.vector.tensor_mul(hseg(mp, a, n), gt[:, :n], pu[:, :n])
                # w2 rides in KI ~0.5 MiB pieces: one monolithic 5.8 MiB DMA
                # head-of-line blocks the critical x/w1 stream at the SDMA
                # engine level (engines drain a whole descriptor batch per
                # queue before switching -- measured: x0's completion
                # semaphore fired at 29 us instead of ~13 us).  Slot 0's
                # pieces additionally carry a scheduler-time floor
                # (tile_wait_until) so the priority heap cannot hoist them
                # into the startup window; emission order alone is NOT
                # preserved by the Tile scheduler for dep-free DMAs.
                if si == 0:
                    with tc.tile_wait_until(0.025 + 0.005 * mp):
                        nc.scalar.dma_start(w2t[:, mp], w2[si, :, mp])
                else:
                    nc.scalar.dma_start(w2t[:, mp], w2[si, :, mp])

            # ---- GEMM2 (tokens on PSUM partitions) ----
            for tb in range(tok // P):
                ci = g0 + (tb * P) // NT
                tbc = (tb * P % NT) // P
                for m in range(NO):
                    po = ps2.tile([P, 512], f32, tag="po")
                    for k in range(KI):
                        nc.tensor.matmul(
                            po[:],
                            hblk(k, tb),
                            w2t[:, k, m * 512 : (m + 1) * 512],
                            start=(k == 0),
                            stop=(k == KI - 1),
                        )
                    om = o_pool.tile([P, 512], bf16, tag="om")
                    nc.vector.tensor_copy(om[:], po[:])
                    nc.scalar.dma_start(out[ci, tbc, m], om[:])
            g0 += sz
    nc.compile()
    return nc


def _get_program(S):
    if S not in _PROGRAM_CACHE:
        _PROGRAM_CACHE[S] = _build_program(S)
    return _PROGRAM_CACHE[S]


# --------------------------------------------------------------------------
# host-side pack / unpack
# --------------------------------------------------------------------------

def _pack_w1(w: np.ndarray) -> np.ndarray:
    # [HIDDEN, GU] f32 -> [KI, P, 2, KH, P] bf16  (h = 128k + p, c = g*INTER
    # + 128*mp + j)
    return np.ascontiguousarray(
        w.reshape(KH, P, 2, KI, P).transpose(3, 1, 2, 0, 4)
    ).astype(BF16)


def _pack_w2(w: np.ndarray) -> np.ndarray:
    # [INTER, HIDDEN] f32 -> [P, KI, HIDDEN] bf16
    return np.ascontiguousarray(
        w.reshape(KI, P, HIDDEN).transpose(1, 0, 2)
    ).astype(BF16)


def _run(
    hidden_states: np.ndarray,
    merged_gate_up_proj: np.ndarray,
    merged_down_proj: np.ndarray,
    num_tokens_per_expert: np.ndarray,
    trace: bool = False,
):
    counts = [int(c) for c in np.asarray(num_tokens_per_expert)]
    n_exp = len(counts)
    offs = np.concatenate([[0], np.cumsum(counts)]).astype(int)
    total = int(offs[-1])

    S, cores = _plan(counts)
    n_slots = len(S)
    n_chunks = sum(S)
    slot_base = np.concatenate([[0], np.cumsum(S)]).astype(int)

    nc = _get_program(S)

    from concurrent.futures import ThreadPoolExecutor

    pool = ThreadPoolExecutor(8)

    # [TOTAL, HIDDEN] f32 -> bf16 -> [P, KH, total] transposed view source
    x_bf16 = hidden_states[:total].astype(BF16)
    xT_full = np.empty((HIDDEN, total), dtype=BF16)

    def _tr(k):
        xT_full[k * P : (k + 1) * P] = x_bf16[:, k * P : (k + 1) * P].T

    list(pool.map(_tr, range(KH)))
    xT_pkt = xT_full.reshape(KH, P, total).transpose(1, 0, 2)  # [P, KH, total]

    w1_packed = list(pool.map(
        lambda e: _pack_w1(merged_gate_up_proj[e]), range(n_exp)
    ))
    w2_packed = list(pool.map(
        lambda e: _pack_w2(merged_down_proj[e]), range(n_exp)
    ))

    def _core_inputs(r):
        xc = np.zeros((n_chunks, P, KH, NT), dtype=BF16)
        w1c = np.empty((n_slots, KI, P, 2, KH, P), dtype=BF16)
        w2c = np.empty((n_slots, P, KI, HIDDEN), dtype=BF16)
        for si in range(n_slots):
            ent = cores[r][si]
            e = ent[0] if ent is not None else 0
            w1c[si] = w1_packed[e]
            w2c[si] = w2_packed[e]
            if ent is None:
                continue
            e, k0, amt = ent
            for j in range(amt):
                t0 = (k0 + j) * NT
                n = min(NT, counts[e] - t0)
                if n <= 0:
                    break
                xc[slot_base[si] + j, :, :, :n] = xT_pkt[
                    :, :, offs[e] + t0 : offs[e] + t0 + n
                ]
        return {"xT": xc, "w1": w1c, "w2": w2c}

    in_maps = list(pool.map(_core_inputs, range(N_CORES)))
    pool.shutdown(wait=True)

    res = _execute(nc, in_maps, trace)

    out = np.empty((total, HIDDEN), dtype=np.float32)

    def _unshard(r):
        o = res.results[r]["out"]  # [n_chunks, TB, NO, P, 512] bf16
        o = o.transpose(0, 1, 3, 2, 4).reshape(n_chunks, NT, HIDDEN)
        for si in range(n_slots):
            ent = cores[r][si]
            if ent is None:
                continue
            e, k0, amt = ent
            for j in range(amt):
                t0 = (k0 + j) * NT
                n = min(NT, counts[e] - t0)
                if n <= 0:
                    break
                out[offs[e] + t0 : offs[e] + t0 + n] = o[
                    slot_base[si] + j, :n
                ].astype(np.float32)

    upool = ThreadPoolExecutor(8)
    list(upool.map(_unshard, range(N_CORES)))
    upool.shutdown(wait=True)
    return out, res


# --------------------------------------------------------------------------
# execution (pjrt fast path with on-device zero outputs, axon fallback)
# --------------------------------------------------------------------------

def _execute(nc, in_maps, trace):
    from concourse.bass_utils import run_bass_kernel_spmd

    if not trace:
        try:
            return _execute_pjrt_dev_zeros(nc, in_maps)
        except Exception:
            pass
    return run_bass_kernel_spmd(
        nc, in_maps, list(range(N_CORES)), trace=trace
    )


_EXEC_CACHE: dict = {}


def _build_pjrt_executor(nc):
    from concourse.bass_utils import axon_active
    import concourse.mybir as mybir
    from concourse import bass2jax
    import jax
    import jax.numpy as jnp
    from jax.sharding import Mesh, PartitionSpec, NamedSharding
    from jax.experimental.shard_map import shard_map

    if not axon_active():
        raise RuntimeError("pjrt path requires axon")
    if nc.dbg_addr is not None:
        raise RuntimeError("debug program")

    bass2jax.install_neuronx_cc_hook()

    partition_name = nc.partition_id_tensor.name if nc.partition_id_tensor else None
    in_names, out_names, out_avals = [], [], []
    for alloc in nc.m.functions[0].allocations:
        if not isinstance(alloc, mybir.MemoryLocationSet):
            continue
        name = alloc.memorylocations[0].name
        if alloc.kind == "ExternalInput":
            if name != partition_name:
                in_names.append(name)
        elif alloc.kind == "ExternalOutput":
            out_names.append(name)
            out_avals.append(
                jax.core.ShapedArray(
                    tuple(alloc.tensor_shape), mybir.dt.np(alloc.dtype)
                )
            )
    n_params = len(in_names)
    n_outs = len(out_avals)
    all_names = in_names + out_names
    if partition_name is not None:
        all_names = all_names + [partition_name]
    donate = tuple(range(n_params, n_params + n_outs))

    def _body(*args):
        operands = list(args)
        if partition_name is not None:
            operands.append(bass2jax.partition_id_tensor())
        outs = bass2jax._bass_exec_p.bind(
            *operands,
            out_avals=tuple(out_avals),
            in_names=tuple(all_names),
            out_names=tuple(out_names),
            lowering_input_output_aliases=(),
            sim_require_finite=True,
            sim_require_nnan=True,
            nc=nc,
        )
        return tuple(outs)

    devices = jax.devices()[:N_CORES]
    assert len(devices) == N_CORES
    mesh = Mesh(np.asarray(devices), ("core",))
    in_specs = (PartitionSpec("core"),) * (n_params + n_outs)
    out_specs = (PartitionSpec("core"),) * n_outs
    sharded = jax.jit(
        shard_map(
            _body, mesh=mesh, in_specs=in_specs, out_specs=out_specs,
            check_rep=False,
        ),
        donate_argnums=donate,
        keep_unused=True,
    )
    zsharding = NamedSharding(mesh, PartitionSpec("core"))
    zero_fns = [
        jax.jit(
            lambda s=av.shape, d=av.dtype: jnp.zeros(
                (N_CORES * s[0], *s[1:]), d
            ),
            out_shardings=zsharding,
        )
        for av in out_avals
    ]
    return {
        "sharded": sharded,
        "zero_fns": zero_fns,
        "in_names": in_names,
        "out_names": out_names,
        "out_avals": out_avals,
    }


def _execute_pjrt_dev_zeros(nc, in_maps):
    """run_bass_via_pjrt equivalent with donated zero output buffers created
    on-device instead of staged from host numpy."""
    from concourse.bass_utils import BassKernelResults

    key = id(nc)
    if key not in _EXEC_CACHE:
        _EXEC_CACHE[key] = _build_pjrt_executor(nc)
    ex = _EXEC_CACHE[key]

    concat_in = [
        np.concatenate([np.asarray(m[name]) for m in in_maps], axis=0)
        for name in ex["in_names"]
    ]
    dev_zeros = [fn() for fn in ex["zero_fns"]]
    out_arrs = ex["sharded"](*concat_in, *dev_zeros)
    out_avals = ex["out_avals"]
    results = [
        {
            name: np.asarray(out_arrs[i]).reshape(
                N_CORES, *out_avals[i].shape
            )[c]
            for i, name in enumerate(ex["out_names"])
        }
        for c in range(N_CORES)
    ]
    return BassKernelResults(
        results=results,
        instructions_and_trace=None,
        profile_json=None,
        exec_time_ns=None,
    )


def kernel(**inputs) -> np.ndarray:
    return _run(**inputs, trace=False)[0]


def run_traced(**inputs):
    return _run(**inputs, trace=True)
